# revision 8
# baseline (speedup 1.0000x reference)
"""Trainium2 Bass kernel for nn_CNFAdapter.

Algorithm (mathematically identical to the reference, heavily folded):

  The literal MLP ``h = gelu([ve[v]; se[s]] @ W1.T + b1) @ W2.T + b2`` only
  has 514 distinct inputs (257 vars x 2 signs), so it is folded on the host
  into a table ``T[514, 256]``.  The clause embedding before LayerNorm is
  ``mean_l h = (1/L) * sum_l T[ci_l]``; dividing T by L bakes in the mean,
  and subtracting each table row's d-mean makes the clause vector exactly
  zero-mean, which removes the LN mean term entirely.

  Per instance the device computes (c = clause, d = hidden, hp = (head,query)):
     xT[d, c]   = tableT @ counts       (counts = per-clause literal histogram)
     rs[c]      = 1/sqrt(sum_d x^2 / D + eps)
     s1v[c, :]  = x @ [Wkq | WvF]       (Wkq folds cn_g, Wk, q, softmax scale)
     expT[c,hp] = exp(rs*s1 + maskbias) (unnormalized softmax, max-sub skipped:
                                         scores are O(1e-2); bk dropped via
                                         softmax shift invariance)
     vq[c,he]   = rs * vtmp             (bv folded into the final bias)
     Z[hp]      = sum_c expT
     bigctx     = vq.T @ expT           (diag head-blocks are the context)
     out        = LN(pqb + ctx @ out_w.T) * pn_g + pn_b

  Sharding: data-parallel over B=32 instances, 4 per NeuronCore; all
  parameters replicated (host-folded, ~1 MB).
"""

import math
from contextlib import ExitStack

import numpy as np

import concourse.bass as bass
import concourse.mybir as mybir
import concourse.tile as tile
from concourse import bacc
from concourse.bass_utils import run_bass_kernel_spmd

# ---------------- problem constants (hardcoded) ----------------
D = 256
H = 8
P = 32
V = 257
EPS = 1e-5
B, C, L = 32, 2048, 8
VOC = 2 * V            # 514 combined (var, sign) literals
VCH = 5                # ceil(514/128) contraction chunks (last has K=2)
NCORES = 8
BPC = B // NCORES      # 4 instances per core
CB = C // 128          # 16 chunks of 128 clauses
hd = D // H

fp16 = mybir.dt.float16
fp32 = mybir.dt.float32
AF = mybir.ActivationFunctionType
ALU = mybir.AluOpType
AX = mybir.AxisListType


def _emit(nc, tc, ctx, dr, out_dram):
    pc = ctx.enter_context(tc.tile_pool(name="consts", bufs=1))
    pcnt = ctx.enter_context(tc.tile_pool(name="cnt", bufs=2))
    px = ctx.enter_context(tc.tile_pool(name="x", bufs=2))
    px2 = ctx.enter_context(tc.tile_pool(name="x2", bufs=2))
    pexp = ctx.enter_context(tc.tile_pool(name="expv", bufs=2))
    pst = ctx.enter_context(tc.tile_pool(name="stats", bufs=2))
    psm = ctx.enter_context(tc.tile_pool(name="small", bufs=2))
    ps_mm = ctx.enter_context(tc.tile_pool(name="ps_mm", bufs=3, space="PSUM"))
    ps_st = ctx.enter_context(tc.tile_pool(name="ps_st", bufs=1, space="PSUM"))
    ps_z = ctx.enter_context(tc.tile_pool(name="ps_z", bufs=1, space="PSUM"))
    ps_bc = ctx.enter_context(tc.tile_pool(name="ps_bc", bufs=1, space="PSUM"))
    ps_tail = ctx.enter_context(tc.tile_pool(name="ps_tail", bufs=1, space="PSUM"))

    # ---- constants to SBUF ----
    tbl = pc.tile([128, VCH, D], fp16, tag="tbl")
    nc.sync.dma_start(out=tbl[:], in_=dr["tbls"][:])
    wkv = pc.tile([128, 2, 2 * D], fp16, tag="wkv")
    nc.sync.dma_start(out=wkv[:], in_=dr["wkv"][:])
    owt = pc.tile([128, 2, D], fp32, tag="owt")
    nc.sync.dma_start(out=owt[:], in_=dr["owt"][:])
    pqb = pc.tile([P, D], fp32, tag="pqb")
    nc.sync.dma_start(out=pqb[:], in_=dr["pqb"][:])
    png = pc.tile([P, D], fp32, tag="png")
    nc.sync.dma_start(out=png[:], in_=dr["png"][:])
    pnb = pc.tile([P, D], fp32, tag="pnb")
    nc.sync.dma_start(out=pnb[:], in_=dr["pnb"][:])
    ones16 = pc.tile([128, 1], fp16, tag="ones16")
    nc.sync.dma_start(out=ones16[:], in_=dr["ones16"][:])
    ident = pc.tile([P, P], fp32, tag="ident")
    nc.sync.dma_start(out=ident[:], in_=dr["ident"][:])
    epst = pc.tile([128, 1], fp32, tag="epst")
    nc.vector.memset(epst[:], EPS)

    for b in range(BPC):
        # ---- load per-instance data ----
        cnt = pcnt.tile([128, VCH, C], fp16, tag="cnt")
        nc.sync.dma_start(out=cnt[:], in_=dr["cnt"][b])
        mb_t = psm.tile([128, CB], fp32, tag="mb")
        nc.sync.dma_start(out=mb_t[:], in_=dr["maskb"][b])

        # ---- xT[d%128, d//128, c] = tableT @ counts (fp32 acc -> fp16) ----
        x = px.tile([128, 2, C], fp16, tag="x")
        for cc in range(4):
            csl = slice(cc * 512, (cc + 1) * 512)
            for dh in range(2):
                pxm = ps_mm.tile([128, 512], fp32, tag="mm512")
                for vc in range(VCH):
                    if vc < 4:
                        lhsT = tbl[:, vc, dh * 128:(dh + 1) * 128]
                        rhs = cnt[:, vc, csl]
                    else:
                        lhsT = tbl[0:2, vc, dh * 128:(dh + 1) * 128]
                        rhs = cnt[0:2, vc, csl]
                    nc.tensor.matmul(pxm[:], lhsT=lhsT, rhs=rhs,
                                     start=(vc == 0), stop=(vc == VCH - 1))
                nc.scalar.copy(x[:, dh, csl], pxm[:])

        # ---- stats: rs[c] = 1/sqrt(sum_d x^2 / D + eps) ----
        x2 = px2.tile([128, 2, C], fp16, tag="x2")
        nc.scalar.square(x2[:], x[:])
        pst_t = ps_st.tile([128, CB], fp32, tag="st")
        for cb in range(CB):
            for dh in range(2):
                nc.tensor.matmul(pst_t[:, cb:cb + 1],
                                 lhsT=x2[:, dh, cb * 128:(cb + 1) * 128],
                                 rhs=ones16[:],
                                 start=(dh == 0), stop=(dh == 1))
        ssq = pst.tile([128, CB], fp32, tag="ssq")
        nc.vector.tensor_copy(ssq[:], pst_t[:])
        sq = pst.tile([128, CB], fp32, tag="sq")
        nc.scalar.activation(sq[:], ssq[:], AF.Sqrt, bias=epst[:, 0:1], scale=1.0 / D)
        rs = pst.tile([128, CB], fp32, tag="rs")
        nc.vector.reciprocal(rs[:], sq[:])

        # ---- s1v = x @ [Wkq | WvF]; fused exp / v-scale epilogues ----
        expT = pexp.tile([128, CB, 256], fp16, tag="expT")
        vq = pexp.tile([128, CB, 256], fp16, tag="vq")
        for cb in range(CB):
            sv = ps_mm.tile([128, 512], fp32, tag="mm512")
            for dh in range(2):
                nc.tensor.matmul(sv[:],
                                 lhsT=x[:, dh, cb * 128:(cb + 1) * 128],
                                 rhs=wkv[:, dh, :],
                                 start=(dh == 0), stop=(dh == 1))
            nc.scalar.activation(expT[:, cb, :], sv[:, 0:256], AF.Exp,
                                 bias=mb_t[:, cb:cb + 1], scale=rs[:, cb:cb + 1])
            nc.scalar.activation(vq[:, cb, :], sv[:, 256:512], AF.Copy,
                                 scale=rs[:, cb:cb + 1])

        # ---- Z and bigctx accumulation over all clauses ----
        pz_t = ps_z.tile([1, 256], fp32, tag="z")
        pbc0 = ps_bc.tile([128, 256], fp32, tag="bc0")
        pbc1 = ps_bc.tile([128, 256], fp32, tag="bc1")
        pbc = [pbc0, pbc1]
        for cb in range(CB):
            nc.tensor.matmul(pz_t[:], lhsT=ones16[:], rhs=expT[:, cb, :],
                             start=(cb == 0), stop=(cb == CB - 1))
            for h2 in range(2):
                nc.tensor.matmul(pbc[h2][:],
                                 lhsT=vq[:, cb, h2 * 128:(h2 + 1) * 128],
                                 rhs=expT[:, cb, :],
                                 start=(cb == 0), stop=(cb == CB - 1))

        # ---- 1/Z in [p, h] layout (32x32 block transpose of the Z row) ----
        zpad = psm.tile([P, 256], fp32, tag="zpad")
        nc.vector.tensor_copy(zpad[0:1, :], pz_t[:])
        zptr = psm.tile([P, 256], fp32, tag="zptr")
        nc.vector.transpose(zptr[:], zpad[:])
        zsel = zptr[:].rearrange("p (h q) -> p h q", q=P)
        zp = psm.tile([P, H], fp32, tag="zp")
        nc.vector.reciprocal(zp[:], zsel[:, :, 0])

        # ---- extract diagonal head blocks, scale by 1/Z -> ctx [p, d] ----
        ctx_t = psm.tile([P, D], fp32, tag="ctx")
        for h in range(H):
            h2, hh = divmod(h, 4)
            blk = pbc[h2][hh * 32:(hh + 1) * 32, h * 32:h * 32 + 32]
            tmp = psm.tile([P, P], fp32, tag="ctxblk")
            nc.vector.transpose(tmp[:], blk)
            nc.vector.tensor_scalar_mul(ctx_t[:, h * 32:(h + 1) * 32], tmp[:],
                                        zp[:, h:h + 1])

        # ---- ctxT (PE transpose) ----
        pct_t = ps_tail.tile([128, 2 * P], fp32, tag="tail")
        for dh in range(2):
            nc.tensor.transpose(pct_t[:, dh * P:(dh + 1) * P],
                                ctx_t[:, dh * 128:(dh + 1) * 128], ident[:])
        ctxT = psm.tile([128, 2 * P], fp32, tag="ctxT")
        nc.scalar.copy(ctxT[:], pct_t[:])

        # ---- refined = ctx @ out_w.T + pqb (pqb via identity matmul) ----
        prf_t = ps_tail.tile([P, D], fp32, tag="tail")
        for dh in range(2):
            nc.tensor.matmul(prf_t[:], lhsT=ctxT[:, dh * P:(dh + 1) * P],
                             rhs=owt[:, dh, :], start=(dh == 0), stop=False)
        nc.tensor.matmul(prf_t[:], lhsT=ident[:], rhs=pqb[:],
                         start=False, stop=True)

        # ---- final LayerNorm ----
        ssum = pst.tile([P, 1], fp32, tag="ssum")
        nc.vector.tensor_reduce(ssum[:], prf_t[:], axis=AX.X, op=ALU.add)
        nm = pst.tile([P, 1], fp32, tag="nm")
        nc.vector.tensor_scalar_mul(nm[:], ssum[:], -1.0 / D)
        cen = psm.tile([P, D], fp32, tag="cen")
        nc.scalar.activation(cen[:], prf_t[:], AF.Identity, bias=nm[:, 0:1])
        sq2 = psm.tile([P, D], fp32, tag="sq2")
        nc.vector.tensor_mul(sq2[:], cen[:], cen[:])
        vs = pst.tile([P, 1], fp32, tag="vs")
        nc.vector.tensor_reduce(vs[:], sq2[:], axis=AX.X, op=ALU.add)
        stdv = pst.tile([P, 1], fp32, tag="stdv")
        nc.scalar.activation(stdv[:], vs[:], AF.Sqrt, bias=epst[0:P, 0:1], scale=1.0 / D)
        rstd = pst.tile([P, 1], fp32, tag="rstd")
        nc.vector.reciprocal(rstd[:], stdv[:])
        t1 = psm.tile([P, D], fp32, tag="t1")
        nc.vector.tensor_scalar_mul(t1[:], cen[:], rstd[:, 0:1])
        t2 = psm.tile([P, D], fp32, tag="t2")
        nc.vector.tensor_mul(t2[:], t1[:], png[:])
        outt = psm.tile([P, D], fp32, tag="outt")
        nc.vector.tensor_add(outt[:], t2[:], pnb[:])
        nc.sync.dma_start(out=out_dram[b], in_=outt[:])


def _build_nc():
    nc = bacc.Bacc("TRN2", target_bir_lowering=False, debug=False,
                   num_devices=NCORES)
    dr = {}

    def din(name, shape, dt):
        dr[name] = nc.dram_tensor(name, shape, dt, kind="ExternalInput").ap()

    din("tbls", [128, VCH, D], fp16)
    din("cnt", [BPC, 128, VCH, C], fp16)
    din("wkv", [128, 2, 2 * D], fp16)
    din("owt", [128, 2, D], fp32)
    din("pqb", [P, D], fp32)
    din("png", [P, D], fp32)
    din("pnb", [P, D], fp32)
    din("maskb", [BPC, 128, CB], fp32)
    din("ones16", [128, 1], fp16)
    din("ident", [P, P], fp32)
    out_dram = nc.dram_tensor("out", [BPC, P, D], fp32, kind="ExternalOutput").ap()

    with tile.TileContext(nc) as tc, ExitStack() as ctx:
        _emit(nc, tc, ctx, dr, out_dram)
    nc.compile()
    return nc


_NC_CACHE = None


def _get_nc():
    global _NC_CACHE
    if _NC_CACHE is None:
        _NC_CACHE = _build_nc()
    return _NC_CACHE


def _erf(x):
    try:
        from scipy.special import erf
        return erf(x)
    except Exception:
        from math import erf as _e
        return np.vectorize(_e)(x)


def host_prepare(inputs):
    """Fold weights, build per-core input maps. All in float64 then cast."""
    ve = inputs["var_embed"].astype(np.float64)
    se = inputs["sign_embed"].astype(np.float64)
    W1 = inputs["W1"].astype(np.float64)
    b1 = inputs["b1"].astype(np.float64)
    W2 = inputs["W2"].astype(np.float64)
    b2 = inputs["b2"].astype(np.float64)
    cn_g = inputs["cn_g"].astype(np.float64)
    cn_b = inputs["cn_b"].astype(np.float64)
    pq = inputs["prefix_queries"].astype(np.float64)
    in_w = inputs["in_proj_w"].astype(np.float64)
    in_b = inputs["in_proj_b"].astype(np.float64)
    out_w = inputs["out_w"].astype(np.float64)
    out_b = inputs["out_b"].astype(np.float64)
    pn_g = inputs["pn_g"].astype(np.float64)
    pn_b = inputs["pn_b"].astype(np.float64)

    # literal table over combined index j = v*2 + s; /L bakes the clause mean,
    # row-centering makes clause vectors exactly zero-mean under LN
    lit = np.concatenate([np.repeat(ve, 2, axis=0), np.tile(se, (V, 1))], axis=1)
    z = lit @ W1.T + b1
    gelu = 0.5 * z * (1.0 + _erf(z / math.sqrt(2.0)))
    table = (gelu @ W2.T + b2) / L
    table = table - table.mean(axis=1, keepdims=True)        # [514, D]
    tpad = np.zeros((VCH * 128, D), np.float64)
    tpad[:VOC] = table
    tbls = np.ascontiguousarray(
        tpad.reshape(VCH, 128, D).transpose(1, 0, 2)).astype(np.float16)

    Wq, Wk, Wv = np.split(in_w, 3, axis=0)
    bq, bk, bv = np.split(in_b, 3)
    q = pq @ Wq.T + bq                                       # [P, D]
    scale = 1.0 / math.sqrt(hd)
    WkF = cn_g[:, None] * Wk.T
    WvF = cn_g[:, None] * Wv.T
    qh = q.reshape(P, H, hd)
    qbd = np.zeros((D, H * P))
    for h in range(H):
        qbd[h * hd:(h + 1) * hd, h * P:(h + 1) * P] = qh[:, h, :].T * scale
    WKV = np.concatenate([WkF @ qbd, WvF], axis=1)           # [D, 512]
    wkv = np.ascontiguousarray(
        WKV.reshape(2, 128, 2 * D).transpose(1, 0, 2)).astype(np.float16)

    bvF = cn_b @ Wv.T + bv                                   # bk dropped (softmax shift)
    pqb = (pq + out_b + bvF @ out_w.T).astype(np.float32)
    owt = np.ascontiguousarray(
        out_w.T.reshape(2, 128, D).transpose(1, 0, 2)).astype(np.float32)

    png = np.broadcast_to(pn_g, (P, D)).astype(np.float32)
    pnb = np.broadcast_to(pn_b, (P, D)).astype(np.float32)
    ident = np.eye(P, dtype=np.float32)
    ones16 = np.ones((128, 1), np.float16)

    # per-clause literal histograms, chunk-transposed: cnt[b, v%128, v//128, c]
    ci = (inputs["var_idx"].astype(np.int64) * 2
          + inputs["sign_idx"].astype(np.int64))             # [B, C, L]
    maskb_full = np.where(np.asarray(inputs["mask"]) > 0, 0.0, -1e9)

    in_maps = []
    for core in range(NCORES):
        cnt = np.zeros((BPC, 128, VCH, C), np.float16)
        mkb = np.zeros((BPC, 128, CB), np.float32)
        for bl in range(BPC):
            bg = core * BPC + bl
            flat = ci[bg].reshape(-1)
            rows = np.repeat(np.arange(C, dtype=np.int64), L)
            cc = np.bincount(rows * (VCH * 128) + flat,
                             minlength=C * VCH * 128).reshape(C, VCH * 128)
            cnt[bl] = cc.reshape(C, VCH, 128).transpose(2, 1, 0)
            mkb[bl] = maskb_full[bg].reshape(CB, 128).T
        in_maps.append({
            "tbls": tbls, "cnt": cnt, "wkv": wkv, "owt": owt, "pqb": pqb,
            "png": png, "pnb": pnb, "maskb": mkb, "ones16": ones16,
            "ident": ident,
        })
    return in_maps


def kernel(**inputs):
    nc = _get_nc()
    in_maps = host_prepare(inputs)
    res = run_bass_kernel_spmd(nc, in_maps, core_ids=list(range(NCORES)))
    out = np.concatenate([res.results[i]["out"] for i in range(NCORES)], axis=0)
    return np.ascontiguousarray(out.astype(np.float32))


# revision 9
# speedup vs baseline: 1.0326x; 1.0326x over previous
"""Trainium2 Bass kernel for nn_CNFAdapter.

Algorithm (mathematically identical to the reference, heavily folded):

  The literal MLP ``h = gelu([ve[v]; se[s]] @ W1.T + b1) @ W2.T + b2`` only
  has 514 distinct inputs (257 vars x 2 signs), so it is folded on the host
  into a table ``T[514, 256]``.  The clause embedding before LayerNorm is
  ``mean_l h = (1/L) * sum_l T[ci_l]``; dividing T by L bakes in the mean,
  and subtracting each table row's d-mean makes the clause vector exactly
  zero-mean, which removes the LN mean term entirely.

  Per instance the device computes (c = clause, d = hidden, hp = (head,query)):
     xT[d, c]   = tableT @ counts       (counts = per-clause literal histogram)
     rs[c]      = 1/sqrt(sum_d x^2 / D + eps)
     s1v[c, :]  = x @ [Wkq | WvF]       (Wkq folds cn_g, Wk, q, softmax scale)
     expT[c,hp] = exp(rs*s1 + maskbias) (unnormalized softmax, max-sub skipped:
                                         scores are O(1e-2); bk dropped via
                                         softmax shift invariance)
     vq[c,he]   = rs * vtmp             (bv folded into the final bias)
     Z[hp]      = sum_c expT
     bigctx     = vq.T @ expT           (diag head-blocks are the context)
     out        = LN(pqb + ctx @ out_w.T) * pn_g + pn_b

  Sharding: data-parallel over B=32 instances, 4 per NeuronCore; all
  parameters replicated (host-folded, ~1 MB).
"""

import math
from contextlib import ExitStack

import numpy as np

import concourse.bass as bass
import concourse.mybir as mybir
import concourse.tile as tile
from concourse import bacc
from concourse.bass_utils import run_bass_kernel_spmd

# ---------------- problem constants (hardcoded) ----------------
D = 256
H = 8
P = 32
V = 257
EPS = 1e-5
B, C, L = 32, 2048, 8
VOC = 2 * V            # 514 combined (var, sign) literals
VCH = 5                # ceil(514/128) contraction chunks (last has K=2)
NCORES = 8
BPC = B // NCORES      # 4 instances per core
CB = C // 128          # 16 chunks of 128 clauses
hd = D // H

fp16 = mybir.dt.float16
fp32 = mybir.dt.float32
AF = mybir.ActivationFunctionType
ALU = mybir.AluOpType
AX = mybir.AxisListType


def _emit(nc, tc, ctx, dr, out_dram):
    pc = ctx.enter_context(tc.tile_pool(name="consts", bufs=1))
    pcnt = ctx.enter_context(tc.tile_pool(name="cnt", bufs=2))
    px = ctx.enter_context(tc.tile_pool(name="x", bufs=2))
    px2 = ctx.enter_context(tc.tile_pool(name="x2", bufs=2))
    pexp = ctx.enter_context(tc.tile_pool(name="expv", bufs=2))
    pst = ctx.enter_context(tc.tile_pool(name="stats", bufs=2))
    psm = ctx.enter_context(tc.tile_pool(name="small", bufs=2))
    ps_mm = ctx.enter_context(tc.tile_pool(name="ps_mm", bufs=3, space="PSUM"))
    ps_st = ctx.enter_context(tc.tile_pool(name="ps_st", bufs=1, space="PSUM"))
    ps_z = ctx.enter_context(tc.tile_pool(name="ps_z", bufs=1, space="PSUM"))
    ps_bc = ctx.enter_context(tc.tile_pool(name="ps_bc", bufs=1, space="PSUM"))
    ps_tail = ctx.enter_context(tc.tile_pool(name="ps_tail", bufs=1, space="PSUM"))

    # ---- constants to SBUF ----
    tbl = pc.tile([128, VCH, D], fp16, tag="tbl")
    nc.sync.dma_start(out=tbl[:], in_=dr["tbls"][:])
    wkv = pc.tile([128, 2, 2 * D], fp16, tag="wkv")
    nc.sync.dma_start(out=wkv[:], in_=dr["wkv"][:])
    owt = pc.tile([128, 2, D], fp32, tag="owt")
    nc.sync.dma_start(out=owt[:], in_=dr["owt"][:])
    pqb = pc.tile([P, D], fp32, tag="pqb")
    nc.sync.dma_start(out=pqb[:], in_=dr["pqb"][:])
    png = pc.tile([P, D], fp32, tag="png")
    nc.sync.dma_start(out=png[:], in_=dr["png"][:])
    pnb = pc.tile([P, D], fp32, tag="pnb")
    nc.sync.dma_start(out=pnb[:], in_=dr["pnb"][:])
    ones16 = pc.tile([128, 1], fp16, tag="ones16")
    nc.sync.dma_start(out=ones16[:], in_=dr["ones16"][:])
    ident = pc.tile([P, P], fp32, tag="ident")
    nc.sync.dma_start(out=ident[:], in_=dr["ident"][:])
    epst = pc.tile([128, 1], fp32, tag="epst")
    nc.vector.memset(epst[:], EPS)

    for b in range(BPC):
        # ---- load per-instance data ----
        cnt = pcnt.tile([128, VCH, C], fp16, tag="cnt")
        nc.sync.dma_start(out=cnt[:], in_=dr["cnt"][b])
        mb_t = psm.tile([128, CB], fp32, tag="mb")
        nc.sync.dma_start(out=mb_t[:], in_=dr["maskb"][b])

        # ---- xT[d%128, d//128, c] = tableT @ counts (fp32 acc -> fp16) ----
        x = px.tile([128, 2, C], fp16, tag="x")
        for cc in range(4):
            csl = slice(cc * 512, (cc + 1) * 512)
            for dh in range(2):
                pxm = ps_mm.tile([128, 512], fp32, tag="mm512")
                for vc in range(VCH):
                    if vc < 4:
                        lhsT = tbl[:, vc, dh * 128:(dh + 1) * 128]
                        rhs = cnt[:, vc, csl]
                    else:
                        lhsT = tbl[0:2, vc, dh * 128:(dh + 1) * 128]
                        rhs = cnt[0:2, vc, csl]
                    nc.tensor.matmul(pxm[:], lhsT=lhsT, rhs=rhs,
                                     start=(vc == 0), stop=(vc == VCH - 1))
                nc.scalar.copy(x[:, dh, csl], pxm[:])

        # ---- stats: rs[c] = 1/sqrt(sum_d x^2 / D + eps) ----
        x2 = px2.tile([128, 2, C], fp16, tag="x2")
        nc.scalar.square(x2[:], x[:])
        pst_t = ps_st.tile([128, CB], fp32, tag="st")
        for cb in range(CB):
            for dh in range(2):
                nc.tensor.matmul(pst_t[:, cb:cb + 1],
                                 lhsT=x2[:, dh, cb * 128:(cb + 1) * 128],
                                 rhs=ones16[:],
                                 start=(dh == 0), stop=(dh == 1))
        ssq = pst.tile([128, CB], fp32, tag="ssq")
        nc.vector.tensor_copy(ssq[:], pst_t[:])
        sq = pst.tile([128, CB], fp32, tag="sq")
        nc.scalar.activation(sq[:], ssq[:], AF.Sqrt, bias=epst[:, 0:1], scale=1.0 / D)
        rs = pst.tile([128, CB], fp32, tag="rs")
        nc.vector.reciprocal(rs[:], sq[:])

        # ---- s1v = x @ [Wkq | WvF]; fused exp / v-scale epilogues ----
        expT = pexp.tile([128, CB, 256], fp16, tag="expT")
        vq = pexp.tile([128, CB, 256], fp16, tag="vq")
        for cb in range(CB):
            sv = ps_mm.tile([128, 512], fp32, tag="mm512")
            for dh in range(2):
                nc.tensor.matmul(sv[:],
                                 lhsT=x[:, dh, cb * 128:(cb + 1) * 128],
                                 rhs=wkv[:, dh, :],
                                 start=(dh == 0), stop=(dh == 1))
            nc.scalar.activation(expT[:, cb, :], sv[:, 0:256], AF.Exp,
                                 bias=mb_t[:, cb:cb + 1], scale=rs[:, cb:cb + 1])
            nc.vector.tensor_scalar_mul(vq[:, cb, :], sv[:, 256:512],
                                        rs[:, cb:cb + 1])

        # ---- Z and bigctx accumulation over all clauses ----
        pz_t = ps_z.tile([1, 256], fp32, tag="z")
        pbc0 = ps_bc.tile([128, 256], fp32, tag="bc0")
        pbc1 = ps_bc.tile([128, 256], fp32, tag="bc1")
        pbc = [pbc0, pbc1]
        for cb in range(CB):
            nc.tensor.matmul(pz_t[:], lhsT=ones16[:], rhs=expT[:, cb, :],
                             start=(cb == 0), stop=(cb == CB - 1))
            for h2 in range(2):
                nc.tensor.matmul(pbc[h2][:],
                                 lhsT=vq[:, cb, h2 * 128:(h2 + 1) * 128],
                                 rhs=expT[:, cb, :],
                                 start=(cb == 0), stop=(cb == CB - 1))

        # ---- 1/Z in [p, h] layout (32x32 block transpose of the Z row) ----
        zpad = psm.tile([P, 256], fp32, tag="zpad")
        nc.vector.tensor_copy(zpad[0:1, :], pz_t[:])
        zptr = psm.tile([P, 256], fp32, tag="zptr")
        nc.vector.transpose(zptr[:], zpad[:])
        zsel = zptr[:].rearrange("p (h q) -> p h q", q=P)
        zp = psm.tile([P, H], fp32, tag="zp")
        nc.vector.reciprocal(zp[:], zsel[:, :, 0])

        # ---- extract diagonal head blocks, scale by 1/Z -> ctx [p, d] ----
        ctx_t = psm.tile([P, D], fp32, tag="ctx")
        for h in range(H):
            h2, hh = divmod(h, 4)
            blk = pbc[h2][hh * 32:(hh + 1) * 32, h * 32:h * 32 + 32]
            tmp = psm.tile([P, P], fp32, tag="ctxblk")
            nc.vector.transpose(tmp[:], blk)
            nc.vector.tensor_scalar_mul(ctx_t[:, h * 32:(h + 1) * 32], tmp[:],
                                        zp[:, h:h + 1])

        # ---- ctxT (PE transpose) ----
        pct_t = ps_tail.tile([128, 2 * P], fp32, tag="tail")
        for dh in range(2):
            nc.tensor.transpose(pct_t[:, dh * P:(dh + 1) * P],
                                ctx_t[:, dh * 128:(dh + 1) * 128], ident[:])
        ctxT = psm.tile([128, 2 * P], fp32, tag="ctxT")
        nc.scalar.copy(ctxT[:], pct_t[:])

        # ---- refined = ctx @ out_w.T + pqb (pqb via identity matmul) ----
        prf_t = ps_tail.tile([P, D], fp32, tag="tail")
        for dh in range(2):
            nc.tensor.matmul(prf_t[:], lhsT=ctxT[:, dh * P:(dh + 1) * P],
                             rhs=owt[:, dh, :], start=(dh == 0), stop=False)
        nc.tensor.matmul(prf_t[:], lhsT=ident[:], rhs=pqb[:],
                         start=False, stop=True)

        # ---- final LayerNorm ----
        ssum = pst.tile([P, 1], fp32, tag="ssum")
        nc.vector.tensor_reduce(ssum[:], prf_t[:], axis=AX.X, op=ALU.add)
        nm = pst.tile([P, 1], fp32, tag="nm")
        nc.vector.tensor_scalar_mul(nm[:], ssum[:], -1.0 / D)
        cen = psm.tile([P, D], fp32, tag="cen")
        nc.scalar.activation(cen[:], prf_t[:], AF.Identity, bias=nm[:, 0:1])
        sq2 = psm.tile([P, D], fp32, tag="sq2")
        nc.vector.tensor_mul(sq2[:], cen[:], cen[:])
        vs = pst.tile([P, 1], fp32, tag="vs")
        nc.vector.tensor_reduce(vs[:], sq2[:], axis=AX.X, op=ALU.add)
        stdv = pst.tile([P, 1], fp32, tag="stdv")
        nc.scalar.activation(stdv[:], vs[:], AF.Sqrt, bias=epst[0:P, 0:1], scale=1.0 / D)
        rstd = pst.tile([P, 1], fp32, tag="rstd")
        nc.vector.reciprocal(rstd[:], stdv[:])
        t1 = psm.tile([P, D], fp32, tag="t1")
        nc.vector.tensor_scalar_mul(t1[:], cen[:], rstd[:, 0:1])
        t2 = psm.tile([P, D], fp32, tag="t2")
        nc.vector.tensor_mul(t2[:], t1[:], png[:])
        outt = psm.tile([P, D], fp32, tag="outt")
        nc.vector.tensor_add(outt[:], t2[:], pnb[:])
        nc.sync.dma_start(out=out_dram[b], in_=outt[:])


def _build_nc():
    nc = bacc.Bacc("TRN2", target_bir_lowering=False, debug=False,
                   num_devices=NCORES)
    dr = {}

    def din(name, shape, dt):
        dr[name] = nc.dram_tensor(name, shape, dt, kind="ExternalInput").ap()

    din("tbls", [128, VCH, D], fp16)
    din("cnt", [BPC, 128, VCH, C], fp16)
    din("wkv", [128, 2, 2 * D], fp16)
    din("owt", [128, 2, D], fp32)
    din("pqb", [P, D], fp32)
    din("png", [P, D], fp32)
    din("pnb", [P, D], fp32)
    din("maskb", [BPC, 128, CB], fp32)
    din("ones16", [128, 1], fp16)
    din("ident", [P, P], fp32)
    out_dram = nc.dram_tensor("out", [BPC, P, D], fp32, kind="ExternalOutput").ap()

    with tile.TileContext(nc) as tc, ExitStack() as ctx:
        _emit(nc, tc, ctx, dr, out_dram)
    nc.compile()
    return nc


_NC_CACHE = None


def _get_nc():
    global _NC_CACHE
    if _NC_CACHE is None:
        _NC_CACHE = _build_nc()
    return _NC_CACHE


def _erf(x):
    try:
        from scipy.special import erf
        return erf(x)
    except Exception:
        from math import erf as _e
        return np.vectorize(_e)(x)


def host_prepare(inputs):
    """Fold weights, build per-core input maps. All in float64 then cast."""
    ve = inputs["var_embed"].astype(np.float64)
    se = inputs["sign_embed"].astype(np.float64)
    W1 = inputs["W1"].astype(np.float64)
    b1 = inputs["b1"].astype(np.float64)
    W2 = inputs["W2"].astype(np.float64)
    b2 = inputs["b2"].astype(np.float64)
    cn_g = inputs["cn_g"].astype(np.float64)
    cn_b = inputs["cn_b"].astype(np.float64)
    pq = inputs["prefix_queries"].astype(np.float64)
    in_w = inputs["in_proj_w"].astype(np.float64)
    in_b = inputs["in_proj_b"].astype(np.float64)
    out_w = inputs["out_w"].astype(np.float64)
    out_b = inputs["out_b"].astype(np.float64)
    pn_g = inputs["pn_g"].astype(np.float64)
    pn_b = inputs["pn_b"].astype(np.float64)

    # literal table over combined index j = v*2 + s; /L bakes the clause mean,
    # row-centering makes clause vectors exactly zero-mean under LN
    lit = np.concatenate([np.repeat(ve, 2, axis=0), np.tile(se, (V, 1))], axis=1)
    z = lit @ W1.T + b1
    gelu = 0.5 * z * (1.0 + _erf(z / math.sqrt(2.0)))
    table = (gelu @ W2.T + b2) / L
    table = table - table.mean(axis=1, keepdims=True)        # [514, D]
    tpad = np.zeros((VCH * 128, D), np.float64)
    tpad[:VOC] = table
    tbls = np.ascontiguousarray(
        tpad.reshape(VCH, 128, D).transpose(1, 0, 2)).astype(np.float16)

    Wq, Wk, Wv = np.split(in_w, 3, axis=0)
    bq, bk, bv = np.split(in_b, 3)
    q = pq @ Wq.T + bq                                       # [P, D]
    scale = 1.0 / math.sqrt(hd)
    WkF = cn_g[:, None] * Wk.T
    WvF = cn_g[:, None] * Wv.T
    qh = q.reshape(P, H, hd)
    qbd = np.zeros((D, H * P))
    for h in range(H):
        qbd[h * hd:(h + 1) * hd, h * P:(h + 1) * P] = qh[:, h, :].T * scale
    WKV = np.concatenate([WkF @ qbd, WvF], axis=1)           # [D, 512]
    wkv = np.ascontiguousarray(
        WKV.reshape(2, 128, 2 * D).transpose(1, 0, 2)).astype(np.float16)

    bvF = cn_b @ Wv.T + bv                                   # bk dropped (softmax shift)
    pqb = (pq + out_b + bvF @ out_w.T).astype(np.float32)
    owt = np.ascontiguousarray(
        out_w.T.reshape(2, 128, D).transpose(1, 0, 2)).astype(np.float32)

    png = np.broadcast_to(pn_g, (P, D)).astype(np.float32)
    pnb = np.broadcast_to(pn_b, (P, D)).astype(np.float32)
    ident = np.eye(P, dtype=np.float32)
    ones16 = np.ones((128, 1), np.float16)

    # per-clause literal histograms, chunk-transposed: cnt[b, v%128, v//128, c]
    ci = (inputs["var_idx"].astype(np.int64) * 2
          + inputs["sign_idx"].astype(np.int64))             # [B, C, L]
    maskb_full = np.where(np.asarray(inputs["mask"]) > 0, 0.0, -1e9)

    in_maps = []
    for core in range(NCORES):
        cnt = np.zeros((BPC, 128, VCH, C), np.float16)
        mkb = np.zeros((BPC, 128, CB), np.float32)
        for bl in range(BPC):
            bg = core * BPC + bl
            flat = ci[bg].reshape(-1)
            rows = np.repeat(np.arange(C, dtype=np.int64), L)
            cc = np.bincount(rows * (VCH * 128) + flat,
                             minlength=C * VCH * 128).reshape(C, VCH * 128)
            cnt[bl] = cc.reshape(C, VCH, 128).transpose(2, 1, 0)
            mkb[bl] = maskb_full[bg].reshape(CB, 128).T
        in_maps.append({
            "tbls": tbls, "cnt": cnt, "wkv": wkv, "owt": owt, "pqb": pqb,
            "png": png, "pnb": pnb, "maskb": mkb, "ones16": ones16,
            "ident": ident,
        })
    return in_maps


def kernel(**inputs):
    nc = _get_nc()
    in_maps = host_prepare(inputs)
    res = run_bass_kernel_spmd(nc, in_maps, core_ids=list(range(NCORES)))
    out = np.concatenate([res.results[i]["out"] for i in range(NCORES)], axis=0)
    return np.ascontiguousarray(out.astype(np.float32))


# revision 10
# speedup vs baseline: 1.0595x; 1.0260x over previous
"""Trainium2 Bass kernel for nn_CNFAdapter.

Algorithm (mathematically identical to the reference, heavily folded):

  The literal MLP ``h = gelu([ve[v]; se[s]] @ W1.T + b1) @ W2.T + b2`` only
  has 514 distinct inputs (257 vars x 2 signs), so it is folded on the host
  into a table ``T[514, 256]``.  The clause embedding before LayerNorm is
  ``mean_l h = (1/L) * sum_l T[ci_l]``; dividing T by L bakes in the mean,
  and subtracting each table row's d-mean makes the clause vector exactly
  zero-mean, which removes the LN mean term entirely.

  Per instance the device computes (c = clause, d = hidden, hp = (head,query)):
     xT[d, c]   = tableT @ counts       (counts = per-clause literal histogram)
     rs[c]      = 1/sqrt(sum_d x^2 / D + eps)
     s1v[c, :]  = x @ [Wkq | WvF]       (Wkq folds cn_g, Wk, q, softmax scale)
     expT[c,hp] = exp(rs*s1 + maskbias) (unnormalized softmax, max-sub skipped:
                                         scores are O(1e-2); bk dropped via
                                         softmax shift invariance)
     vq[c,he]   = rs * vtmp             (bv folded into the final bias)
     Z[hp]      = sum_c expT
     bigctx     = vq.T @ expT           (diag head-blocks are the context)
     out        = LN(pqb + ctx @ out_w.T) * pn_g + pn_b

  Sharding: data-parallel over B=32 instances, 4 per NeuronCore; all
  parameters replicated (host-folded, ~1 MB).
"""

import math
from contextlib import ExitStack

import numpy as np

import concourse.bass as bass
import concourse.mybir as mybir
import concourse.tile as tile
from concourse import bacc
from concourse.bass_utils import run_bass_kernel_spmd

# ---------------- problem constants (hardcoded) ----------------
D = 256
H = 8
P = 32
V = 257
EPS = 1e-5
B, C, L = 32, 2048, 8
VOC = 2 * V            # 514 combined (var, sign) literals
VCH = 5                # ceil(514/128) contraction chunks (last has K=2)
NCORES = 8
BPC = B // NCORES      # 4 instances per core
CB = C // 128          # 16 chunks of 128 clauses
hd = D // H

fp16 = mybir.dt.float16
fp32 = mybir.dt.float32
AF = mybir.ActivationFunctionType
ALU = mybir.AluOpType
AX = mybir.AxisListType


def _emit(nc, tc, ctx, dr, out_dram):
    pc = ctx.enter_context(tc.tile_pool(name="consts", bufs=1))
    pcnt = ctx.enter_context(tc.tile_pool(name="cnt", bufs=4))
    px = ctx.enter_context(tc.tile_pool(name="x", bufs=2))
    px2 = ctx.enter_context(tc.tile_pool(name="x2", bufs=2))
    pexp = ctx.enter_context(tc.tile_pool(name="expv", bufs=2))
    pst = ctx.enter_context(tc.tile_pool(name="stats", bufs=2))
    psm = ctx.enter_context(tc.tile_pool(name="small", bufs=2))
    ps_mm = ctx.enter_context(tc.tile_pool(name="ps_mm", bufs=3, space="PSUM"))
    ps_st = ctx.enter_context(tc.tile_pool(name="ps_st", bufs=1, space="PSUM"))
    ps_z = ctx.enter_context(tc.tile_pool(name="ps_z", bufs=1, space="PSUM"))
    ps_bc = ctx.enter_context(tc.tile_pool(name="ps_bc", bufs=1, space="PSUM"))
    ps_tail = ctx.enter_context(tc.tile_pool(name="ps_tail", bufs=1, space="PSUM"))

    # ---- constants to SBUF ----
    tbl = pc.tile([128, VCH, D], fp16, tag="tbl")
    nc.sync.dma_start(out=tbl[:], in_=dr["tbls"][:])
    wkv = pc.tile([128, 2, 2 * D], fp16, tag="wkv")
    nc.sync.dma_start(out=wkv[:], in_=dr["wkv"][:])
    owt = pc.tile([128, 2, D], fp32, tag="owt")
    nc.sync.dma_start(out=owt[:], in_=dr["owt"][:])
    pqb = pc.tile([P, D], fp32, tag="pqb")
    nc.sync.dma_start(out=pqb[:], in_=dr["pqb"][:])
    png = pc.tile([P, D], fp32, tag="png")
    nc.sync.dma_start(out=png[:], in_=dr["png"][:])
    pnb = pc.tile([P, D], fp32, tag="pnb")
    nc.sync.dma_start(out=pnb[:], in_=dr["pnb"][:])
    ones16 = pc.tile([128, 1], fp16, tag="ones16")
    nc.sync.dma_start(out=ones16[:], in_=dr["ones16"][:])
    ident = pc.tile([P, P], fp32, tag="ident")
    nc.sync.dma_start(out=ident[:], in_=dr["ident"][:])
    epst = pc.tile([128, 1], fp32, tag="epst")
    nc.vector.memset(epst[:], EPS)

    for b in range(BPC):
        # ---- load per-instance data ----
        cnt = pcnt.tile([128, VCH, C], fp16, tag="cnt")
        nc.sync.dma_start(out=cnt[:], in_=dr["cnt"][b])
        mb_t = psm.tile([128, CB], fp32, tag="mb")
        nc.sync.dma_start(out=mb_t[:], in_=dr["maskb"][b])

        # ---- xT[d%128, d//128, c] = tableT @ counts (fp32 acc -> fp16) ----
        x = px.tile([128, 2, C], fp16, tag="x")
        for cc in range(4):
            csl = slice(cc * 512, (cc + 1) * 512)
            for dh in range(2):
                pxm = ps_mm.tile([128, 512], fp32, tag="mm512")
                for vc in range(VCH):
                    if vc < 4:
                        lhsT = tbl[:, vc, dh * 128:(dh + 1) * 128]
                        rhs = cnt[:, vc, csl]
                    else:
                        lhsT = tbl[0:2, vc, dh * 128:(dh + 1) * 128]
                        rhs = cnt[0:2, vc, csl]
                    nc.tensor.matmul(pxm[:], lhsT=lhsT, rhs=rhs,
                                     start=(vc == 0), stop=(vc == VCH - 1))
                nc.scalar.copy(x[:, dh, csl], pxm[:])

        # ---- stats: rs[c] = 1/sqrt(sum_d x^2 / D + eps) ----
        x2 = px2.tile([128, 2, C], fp16, tag="x2")
        nc.scalar.square(x2[:], x[:])
        pst_t = ps_st.tile([128, CB], fp32, tag="st")
        for cb in range(CB):
            for dh in range(2):
                nc.tensor.matmul(pst_t[:, cb:cb + 1],
                                 lhsT=x2[:, dh, cb * 128:(cb + 1) * 128],
                                 rhs=ones16[:],
                                 start=(dh == 0), stop=(dh == 1))
        ssq = pst.tile([128, CB], fp32, tag="ssq")
        nc.vector.tensor_copy(ssq[:], pst_t[:])
        sq = pst.tile([128, CB], fp32, tag="sq")
        nc.scalar.activation(sq[:], ssq[:], AF.Sqrt, bias=epst[:, 0:1], scale=1.0 / D)
        rs = pst.tile([128, CB], fp32, tag="rs")
        nc.vector.reciprocal(rs[:], sq[:])

        # ---- s1v = x @ [Wkq | WvF]; fused exp / v-scale epilogues ----
        expT = pexp.tile([128, CB, 256], fp16, tag="expT")
        vq = pexp.tile([128, CB, 256], fp16, tag="vq")
        for cb in range(CB):
            sv = ps_mm.tile([128, 512], fp32, tag="mm512")
            for dh in range(2):
                nc.tensor.matmul(sv[:],
                                 lhsT=x[:, dh, cb * 128:(cb + 1) * 128],
                                 rhs=wkv[:, dh, :],
                                 start=(dh == 0), stop=(dh == 1))
            nc.scalar.activation(expT[:, cb, :], sv[:, 0:256], AF.Exp,
                                 bias=mb_t[:, cb:cb + 1], scale=rs[:, cb:cb + 1])
            nc.vector.tensor_scalar_mul(vq[:, cb, :], sv[:, 256:512],
                                        rs[:, cb:cb + 1])

        # ---- Z and bigctx accumulation over all clauses ----
        pz_t = ps_z.tile([1, 256], fp32, tag="z")
        pbc0 = ps_bc.tile([128, 256], fp32, tag="bc0")
        pbc1 = ps_bc.tile([128, 256], fp32, tag="bc1")
        pbc = [pbc0, pbc1]
        for cb in range(CB):
            nc.tensor.matmul(pz_t[:], lhsT=ones16[:], rhs=expT[:, cb, :],
                             start=(cb == 0), stop=(cb == CB - 1))
            for h2 in range(2):
                nc.tensor.matmul(pbc[h2][:],
                                 lhsT=vq[:, cb, h2 * 128:(h2 + 1) * 128],
                                 rhs=expT[:, cb, :],
                                 start=(cb == 0), stop=(cb == CB - 1))

        # ---- 1/Z in [p, h] layout (32x32 block transpose of the Z row) ----
        zpad = psm.tile([P, 256], fp32, tag="zpad")
        nc.vector.tensor_copy(zpad[0:1, :], pz_t[:])
        zptr = psm.tile([P, 256], fp32, tag="zptr")
        nc.vector.transpose(zptr[:], zpad[:])
        zsel = zptr[:].rearrange("p (h q) -> p h q", q=P)
        zp = psm.tile([P, H], fp32, tag="zp")
        nc.vector.reciprocal(zp[:], zsel[:, :, 0])

        # ---- extract diagonal head blocks, scale by 1/Z -> ctx [p, d] ----
        ctx_t = psm.tile([P, D], fp32, tag="ctx")
        for h in range(H):
            h2, hh = divmod(h, 4)
            blk = pbc[h2][hh * 32:(hh + 1) * 32, h * 32:h * 32 + 32]
            tmp = psm.tile([P, P], fp32, tag="ctxblk")
            nc.vector.transpose(tmp[:], blk)
            nc.vector.tensor_scalar_mul(ctx_t[:, h * 32:(h + 1) * 32], tmp[:],
                                        zp[:, h:h + 1])

        # ---- ctxT (PE transpose) ----
        pct_t = ps_tail.tile([128, 2 * P], fp32, tag="tail")
        for dh in range(2):
            nc.tensor.transpose(pct_t[:, dh * P:(dh + 1) * P],
                                ctx_t[:, dh * 128:(dh + 1) * 128], ident[:])
        ctxT = psm.tile([128, 2 * P], fp32, tag="ctxT")
        nc.scalar.copy(ctxT[:], pct_t[:])

        # ---- refined = ctx @ out_w.T + pqb (pqb via identity matmul) ----
        prf_t = ps_tail.tile([P, D], fp32, tag="tail")
        for dh in range(2):
            nc.tensor.matmul(prf_t[:], lhsT=ctxT[:, dh * P:(dh + 1) * P],
                             rhs=owt[:, dh, :], start=(dh == 0), stop=False)
        nc.tensor.matmul(prf_t[:], lhsT=ident[:], rhs=pqb[:],
                         start=False, stop=True)

        # ---- final LayerNorm ----
        ssum = pst.tile([P, 1], fp32, tag="ssum")
        nc.vector.tensor_reduce(ssum[:], prf_t[:], axis=AX.X, op=ALU.add)
        nm = pst.tile([P, 1], fp32, tag="nm")
        nc.vector.tensor_scalar_mul(nm[:], ssum[:], -1.0 / D)
        cen = psm.tile([P, D], fp32, tag="cen")
        nc.scalar.activation(cen[:], prf_t[:], AF.Identity, bias=nm[:, 0:1])
        sq2 = psm.tile([P, D], fp32, tag="sq2")
        nc.vector.tensor_mul(sq2[:], cen[:], cen[:])
        vs = pst.tile([P, 1], fp32, tag="vs")
        nc.vector.tensor_reduce(vs[:], sq2[:], axis=AX.X, op=ALU.add)
        stdv = pst.tile([P, 1], fp32, tag="stdv")
        nc.scalar.activation(stdv[:], vs[:], AF.Sqrt, bias=epst[0:P, 0:1], scale=1.0 / D)
        rstd = pst.tile([P, 1], fp32, tag="rstd")
        nc.vector.reciprocal(rstd[:], stdv[:])
        t1 = psm.tile([P, D], fp32, tag="t1")
        nc.vector.tensor_scalar_mul(t1[:], cen[:], rstd[:, 0:1])
        t2 = psm.tile([P, D], fp32, tag="t2")
        nc.vector.tensor_mul(t2[:], t1[:], png[:])
        outt = psm.tile([P, D], fp32, tag="outt")
        nc.vector.tensor_add(outt[:], t2[:], pnb[:])
        nc.sync.dma_start(out=out_dram[b], in_=outt[:])


def _build_nc():
    nc = bacc.Bacc("TRN2", target_bir_lowering=False, debug=False,
                   num_devices=NCORES)
    dr = {}

    def din(name, shape, dt):
        dr[name] = nc.dram_tensor(name, shape, dt, kind="ExternalInput").ap()

    din("tbls", [128, VCH, D], fp16)
    din("cnt", [BPC, 128, VCH, C], fp16)
    din("wkv", [128, 2, 2 * D], fp16)
    din("owt", [128, 2, D], fp32)
    din("pqb", [P, D], fp32)
    din("png", [P, D], fp32)
    din("pnb", [P, D], fp32)
    din("maskb", [BPC, 128, CB], fp32)
    din("ones16", [128, 1], fp16)
    din("ident", [P, P], fp32)
    out_dram = nc.dram_tensor("out", [BPC, P, D], fp32, kind="ExternalOutput").ap()

    with tile.TileContext(nc) as tc, ExitStack() as ctx:
        _emit(nc, tc, ctx, dr, out_dram)
    nc.compile()
    return nc


_NC_CACHE = None


def _get_nc():
    global _NC_CACHE
    if _NC_CACHE is None:
        _NC_CACHE = _build_nc()
    return _NC_CACHE


def _erf(x):
    try:
        from scipy.special import erf
        return erf(x)
    except Exception:
        from math import erf as _e
        return np.vectorize(_e)(x)


def host_prepare(inputs):
    """Fold weights, build per-core input maps. All in float64 then cast."""
    ve = inputs["var_embed"].astype(np.float64)
    se = inputs["sign_embed"].astype(np.float64)
    W1 = inputs["W1"].astype(np.float64)
    b1 = inputs["b1"].astype(np.float64)
    W2 = inputs["W2"].astype(np.float64)
    b2 = inputs["b2"].astype(np.float64)
    cn_g = inputs["cn_g"].astype(np.float64)
    cn_b = inputs["cn_b"].astype(np.float64)
    pq = inputs["prefix_queries"].astype(np.float64)
    in_w = inputs["in_proj_w"].astype(np.float64)
    in_b = inputs["in_proj_b"].astype(np.float64)
    out_w = inputs["out_w"].astype(np.float64)
    out_b = inputs["out_b"].astype(np.float64)
    pn_g = inputs["pn_g"].astype(np.float64)
    pn_b = inputs["pn_b"].astype(np.float64)

    # literal table over combined index j = v*2 + s; /L bakes the clause mean,
    # row-centering makes clause vectors exactly zero-mean under LN
    lit = np.concatenate([np.repeat(ve, 2, axis=0), np.tile(se, (V, 1))], axis=1)
    z = lit @ W1.T + b1
    gelu = 0.5 * z * (1.0 + _erf(z / math.sqrt(2.0)))
    table = (gelu @ W2.T + b2) / L
    table = table - table.mean(axis=1, keepdims=True)        # [514, D]
    tpad = np.zeros((VCH * 128, D), np.float64)
    tpad[:VOC] = table
    tbls = np.ascontiguousarray(
        tpad.reshape(VCH, 128, D).transpose(1, 0, 2)).astype(np.float16)

    Wq, Wk, Wv = np.split(in_w, 3, axis=0)
    bq, bk, bv = np.split(in_b, 3)
    q = pq @ Wq.T + bq                                       # [P, D]
    scale = 1.0 / math.sqrt(hd)
    WkF = cn_g[:, None] * Wk.T
    WvF = cn_g[:, None] * Wv.T
    qh = q.reshape(P, H, hd)
    qbd = np.zeros((D, H * P))
    for h in range(H):
        qbd[h * hd:(h + 1) * hd, h * P:(h + 1) * P] = qh[:, h, :].T * scale
    WKV = np.concatenate([WkF @ qbd, WvF], axis=1)           # [D, 512]
    wkv = np.ascontiguousarray(
        WKV.reshape(2, 128, 2 * D).transpose(1, 0, 2)).astype(np.float16)

    bvF = cn_b @ Wv.T + bv                                   # bk dropped (softmax shift)
    pqb = (pq + out_b + bvF @ out_w.T).astype(np.float32)
    owt = np.ascontiguousarray(
        out_w.T.reshape(2, 128, D).transpose(1, 0, 2)).astype(np.float32)

    png = np.broadcast_to(pn_g, (P, D)).astype(np.float32)
    pnb = np.broadcast_to(pn_b, (P, D)).astype(np.float32)
    ident = np.eye(P, dtype=np.float32)
    ones16 = np.ones((128, 1), np.float16)

    # per-clause literal histograms, chunk-transposed: cnt[b, v%128, v//128, c]
    ci = (inputs["var_idx"].astype(np.int64) * 2
          + inputs["sign_idx"].astype(np.int64))             # [B, C, L]
    maskb_full = np.where(np.asarray(inputs["mask"]) > 0, 0.0, -1e9)

    in_maps = []
    for core in range(NCORES):
        cnt = np.zeros((BPC, 128, VCH, C), np.float16)
        mkb = np.zeros((BPC, 128, CB), np.float32)
        for bl in range(BPC):
            bg = core * BPC + bl
            flat = ci[bg].reshape(-1)
            rows = np.repeat(np.arange(C, dtype=np.int64), L)
            cc = np.bincount(rows * (VCH * 128) + flat,
                             minlength=C * VCH * 128).reshape(C, VCH * 128)
            cnt[bl] = cc.reshape(C, VCH, 128).transpose(2, 1, 0)
            mkb[bl] = maskb_full[bg].reshape(CB, 128).T
        in_maps.append({
            "tbls": tbls, "cnt": cnt, "wkv": wkv, "owt": owt, "pqb": pqb,
            "png": png, "pnb": pnb, "maskb": mkb, "ones16": ones16,
            "ident": ident,
        })
    return in_maps


def kernel(**inputs):
    nc = _get_nc()
    in_maps = host_prepare(inputs)
    res = run_bass_kernel_spmd(nc, in_maps, core_ids=list(range(NCORES)))
    out = np.concatenate([res.results[i]["out"] for i in range(NCORES)], axis=0)
    return np.ascontiguousarray(out.astype(np.float32))
